# revision 45
# baseline (speedup 1.0000x reference)
"""Trainium2 Bass kernel for nn_BandSplitDCTFilter.

Math: the reference's mirror-FFT DCT / band filter / inverse collapses to
    out_c = C1 (Z_c) C2^T - S1 (Z_c) S2^T,   Z_c = (A x_c A^T) .* W_eff_c
with A[k,j] = 2cos(pi k (2j+1)/128); C2/S2 carry the irfft half-spectrum
weights u_l and the 1/(4HW) scale; W_eff = pad(W_low)+pad(W_mid)+W_high
merges the three bands (they share the inverse basis under zero-padding).
Then y = x_out @ proj_w^T and LayerNorm.

Sharding: pure data-parallel, one sample per core (B=8 = 8 cores), small
weights replicated.

Final version (~107us HW vs the 121us two-pipe v1 and the 133us graded
baseline).  Changes over v1, driven by the measured DMA cost model
(every dma_start costs its ISSUING sequencer ~0.7us + ~1.7ns/descriptor
as a DIRECT2D instruction, plus ~30-44ns/descriptor on the 16 SDMA
queues; scattered 256B-run transfers are descriptor-bound):
  - pipe A keeps the sync queue; ALL of pipe B's DMAs move to the
    gpsimd queue so the scalar (ACT) sequencer never issues DMAs --
    in v1 pipe B's descriptor generation stole ~20us of ACT compute.
  - both pivots now store CONTIGUOUSLY to DRAM (cheap, few descriptors)
    and do the transpose on the gather-read side: p1 reloads per
    khh-half so s4 starts after half the gather, p2 reloads into four
    per-quarter Ustk tiles so s7/proj/LN pipeline against the gather.
    (SBUF->SBUF pivots would be ~2x cheaper still, but the walrus BIR
    verifier requires partition-crossing SBUF access patterns to be
    tile-anchored at offset 0, which the 32-row chunked sources here
    cannot satisfy -- hence DRAM-mediated exchanges.)
  - DMA instruction count ~60 -> ~36: packed consts (khbd|cs2|ics|pjt
    in one [128,832] tensor), one x load per pipe, merged pivot dumps.
  - LN smalls per group of 8; normalize runs 7/8 on gpsimd (tensor_
    scalar in SBUF) to unload the vector engine, which saturated the
    tail; X01 drains lean 3/4 onto scalar for the same reason.
  - PSUM split: 5-deep [128,512] ring for transform stages + 3-deep
    [128,256] ring for proj tiles.
"""

import os

os.environ.setdefault("JAX_PLATFORMS", "axon,cpu")

import numpy as np
import ml_dtypes

import bass_rust
import concourse.bass as bass
import concourse.mybir as mybir
from concourse.tile import TileContext, ScopedClock
from concourse.bass_utils import run_bass_kernel_spmd

# ---------------------------------------------------------------------------
# Workarounds: this container's walrus rejects >1 sync wait per instruction.
# ---------------------------------------------------------------------------

_wait_ctr = 0


def _split_multi_waits(nc, max_waits=1):
    global _wait_ctr
    for f in nc.m.functions:
        for bb in f.blocks:
            out = []
            dirty = False
            for ins in bb.instructions:
                si = ins.sync_info
                if si is not None and len(si.on_wait) > max_waits:
                    waits = list(si.on_wait)
                    for w in waits[:-max_waits]:
                        _wait_ctr += 1
                        nop = bass_rust.InstNoOp(name=f"I-waitsplit-{_wait_ctr}")
                        nop.engine = ins.engine
                        nop.sync_info = mybir.SyncInfo(on_wait=[w], on_update=[])
                        out.append(nop)
                    ins.sync_info = mybir.SyncInfo(
                        on_wait=waits[-max_waits:], on_update=list(si.on_update)
                    )
                    dirty = True
                out.append(ins)
            if dirty:
                bb.instructions = out


def _patched_drain_and_barrier(self, tick_clock, wait_clock):
    nc = self.nc
    probe = nc.sync.nop(nofuse=True)
    wait_clock.add_sem_waits(probe.ins, ScopedClock({None: tick_clock.global_clock}))
    si = probe.ins.sync_info
    waits = list(si.on_wait) if si is not None else []
    probe.ins.sync_info = mybir.SyncInfo(on_wait=waits[:1], on_update=[])
    name2sem = {s.name: s for s in self.sems.allocated().values()}
    for w in waits[1:]:
        nc.sync.nop(nofuse=True)._wait_ge(name2sem[w.ant_name], w.wait_value)
    nc.sync.drain()
    nc.all_engine_barrier()
    popped = nc._tile_sem_poison_stack.pop()
    assert popped is self._sem_poison
    nc.clear_and_free_semaphores(list(self.sems.allocated().values()))
    nc.all_engine_barrier()


TileContext._drain_and_barrier = _patched_drain_and_barrier

# ---------------------------------------------------------------------------

B, H, W, C = 8, 64, 64, 256
N = H * W
F32 = mybir.dt.float32
BF16 = mybir.dt.bfloat16
ALU = mybir.AluOpType
ACTF = mybir.ActivationFunctionType


def _host_matrices():
    k = np.arange(64)
    j = np.arange(64)
    ang = np.pi * k[:, None] * (2 * j[None, :] + 1) / 128.0
    A = 2.0 * np.cos(ang)
    u = np.where(k == 0, 1.0, 2.0)
    C1T = np.cos(ang)
    S1T = np.sin(ang)
    C2T = u[:, None] * np.cos(ang) / 16384.0
    S2T = u[:, None] * np.sin(ang) / 16384.0

    AT = A.T.astype(np.float32)                                   # [h, k]
    khbd = np.zeros((128, 128), np.float32)
    khbd[0:64, 0:64] = AT
    khbd[64:128, 64:128] = AT
    cs2_half = np.concatenate([C2T, S2T], axis=1)                 # [l, 128]
    cs2 = np.concatenate([cs2_half, cs2_half], axis=0)
    ICS = np.concatenate([C1T, -S1T], axis=0)
    return (khbd.astype(ml_dtypes.bfloat16),
            cs2.astype(ml_dtypes.bfloat16),
            np.ascontiguousarray(ICS.astype(ml_dtypes.bfloat16)))


_NC_CACHE = {}


def _build_nc(apply_gb):
    nc = bass.Bass(trn_type="TRN2")

    xa_d = nc.dram_tensor("xra", [128, 4096], BF16, kind="ExternalInput")
    xb_d = nc.dram_tensor("xrb", [128, 4096], BF16, kind="ExternalInput")
    cst_d = nc.dram_tensor("cst", [128, 832], BF16, kind="ExternalInput")
    wa_d = nc.dram_tensor("weffa", [128, 4096], BF16, kind="ExternalInput")
    wb_d = nc.dram_tensor("weffb", [128, 4096], BF16, kind="ExternalInput")
    gb_d = nc.dram_tensor("gb", [2, 256], F32, kind="ExternalInput")
    y_d = nc.dram_tensor("y", [128, 8192], BF16, kind="ExternalOutput")

    with TileContext(nc) as tc:
        with (
            tc.tile_pool(name="consts", bufs=1) as consts,
            tc.tile_pool(name="wfA", bufs=1) as wfA,
            tc.tile_pool(name="wfB", bufs=1) as wfB,
            tc.tile_pool(name="sA1", bufs=1) as sA1,
            tc.tile_pool(name="sA2", bufs=1) as sA2,
            tc.tile_pool(name="sA3", bufs=1) as sA3,
            tc.tile_pool(name="sB1", bufs=1) as sB1,
            tc.tile_pool(name="sB2", bufs=1) as sB2,
            tc.tile_pool(name="sB3", bufs=1) as sB3,
            tc.tile_pool(name="zA", bufs=1) as zA,
            tc.tile_pool(name="zB", bufs=1) as zB,
            tc.tile_pool(name="yr", bufs=1) as yr,
            tc.tile_pool(name="dramp", bufs=1, space="DRAM") as dramp,
            tc.tile_pool(name="ps", bufs=5, space="PSUM") as ps,
            tc.tile_pool(name="psy", bufs=3, space="PSUM") as psy,
            tc.tile_pool(name="small", bufs=16) as small,
        ):
            # ---- constants (one packed dma on gpsimd) ----
            cst = consts.tile([128, 832], BF16, tag="cst")
            nc.gpsimd.dma_start(out=cst[:], in_=cst_d[:])
            khbd = cst[:, 0:128]
            cs2 = cst[:, 128:256]
            ics = cst[:, 256:320]
            pjt = cst[:, 320:832]
            eps = consts.tile([128, 1], F32, tag="eps")
            nc.vector.memset(eps[:], 1e-5)
            i256 = consts.tile([128, 1], F32, tag="i256")
            nc.vector.memset(i256[:], 1.0 / 256.0)
            weffA = wfA.tile([128, 4096], BF16, tag="wfA")
            weffB = wfB.tile([128, 4096], BF16, tag="wfB")
            # NOTE: weff dmas are issued AFTER the x loads (below) -- the
            # gpsimd ring is FIFO, and 2MB of weff ahead of pipe B's x load
            # delayed s2(B) by ~5us (weff isn't needed until s4).
            if apply_gb:
                gt = consts.tile([128, 256], F32, tag="gt")
                bt = consts.tile([128, 256], F32, tag="bt")
                gb_ap = gb_d.ap()
                g_b = bass.AP(tensor=gb_ap.tensor, offset=0, ap=[[0, 128], [1, 256]])
                b_b = bass.AP(tensor=gb_ap.tensor, offset=256, ap=[[0, 128], [1, 256]])
                nc.gpsimd.dma_start(out=gt[:], in_=g_b)
                nc.gpsimd.dma_start(out=bt[:], in_=b_b)

            cfg = {
                0: dict(x_d=xa_d, io=nc.sync, s1=sA1, s2=sA2, s3=sA3, zp=zA),
                1: dict(x_d=xb_d, io=nc.gpsimd, s1=sB1, s2=sB2, s3=sB3, zp=zB),
            }
            st = {0: {}, 1: {}}

            def s1_load(P):
                c = cfg[P]
                X = c["s1"].tile([128, 4096], BF16, tag=f"s{P}1")
                c["io"].dma_start(out=X[:], in_=c["x_d"][:])
                st[P]["X"] = X

            def s2_fh(P):
                # T1[(wh,k),(w32,c)] = blockdiag(A^T)^T @ X  (K=128 full)
                c = cfg[P]
                X = st[P]["X"]
                T1p = c["s2"].tile([128, 4096], BF16, tag=f"s{P}2")
                for j in range(8):
                    sl = slice(j * 512, (j + 1) * 512)
                    pt = ps.tile([128, 512], F32, tag="ps")
                    nc.tensor.matmul(pt[:], khbd, X[:, sl],
                                     start=True, stop=True)
                    eng = nc.vector.tensor_copy if j % 2 == 0 else nc.scalar.copy
                    eng(T1p[:, sl], pt[:])
                st[P]["T1p"] = T1p

            def p1_pivot(P):
                # Contiguous dump of T1p rows; the kh<->w exchange happens on
                # the READ side so s4 can start per khh-half.
                c = cfg[P]
                T1p = st[P]["T1p"]
                D1 = dramp.tile([128, 4096], BF16, tag=f"d1{P}", name=f"D1_{P}")
                c["io"].dma_start(out=D1[:], in_=T1p[:])
                T2h = [c["s3"].tile([128, 2048], BF16, tag=f"t2{P}{h}",
                                    name=f"T2h{P}{h}") for h in range(2)]
                for h in range(2):
                    for ks in range(2):
                        for w1 in range(2):
                            dst = T2h[h][ks * 64 + w1 * 32:
                                         ks * 64 + w1 * 32 + 32, :]
                            r0 = w1 * 64 + ks * 32 + h * 16
                            src = D1[r0: r0 + 16, :].rearrange(
                                "k (w c) -> w k c", c=128)
                            # pipe A: halve the serial descriptor-gen on the
                            # sync sequencer (ACT's is idle in this window)
                            eng = (nc.scalar if (P == 0 and ks == 1)
                                   else c["io"])
                            eng.dma_start(
                                out=dst.rearrange("w (k c) -> w k c", c=128),
                                in_=src)
                st[P]["T2h"] = T2h

            def s4_s5(P):
                c = cfg[P]
                T2h = st[P]["T2h"]
                weff = weffA if P == 0 else weffB
                Zp = c["zp"].tile([128, 4096], BF16, tag=f"z{P}")
                for j in range(8):
                    sl = slice(j * 512, (j + 1) * 512)
                    pt = ps.tile([128, 512], F32, tag="ps")
                    nc.tensor.matmul(pt[:], khbd,
                                     T2h[j // 4][:, (j % 4) * 512:
                                                 (j % 4 + 1) * 512],
                                     start=True, stop=True)
                    nc.vector.tensor_mul(Zp[:, sl], pt[:], weff[:, sl])
                U2s = c["s3"].tile([128, 8192], BF16, tag=f"s{P}3")
                for j in range(16):
                    off = 64 * (j // 8)
                    sl = slice((j % 8) * 512, (j % 8 + 1) * 512)
                    pt = ps.tile([128, 512], F32, tag="ps")
                    nc.tensor.matmul(pt[:], cs2[off:off + 64, :],
                                     Zp[off:off + 64, sl], start=True, stop=True)
                    dsl = slice(j * 512, (j + 1) * 512)
                    eng = nc.vector.tensor_copy if j % 2 == 0 else nc.scalar.copy
                    eng(U2s[:, dsl], pt[:])
                st[P]["U2s"] = U2s

            def p2_pivot(P):
                # Contiguous store of U2s; the j<->kh exchange happens on the
                # READ side (gather loads) so it pipelines with s7/proj.
                c = cfg[P]
                U2s = st[P]["U2s"]
                D2 = dramp.tile([128, 8192], BF16, tag=f"d2{P}", name=f"D2_{P}")
                c["io"].dma_start(out=D2[:], in_=U2s[:])
                Usq = [c["s1"].tile([128, 2048], BF16, tag=f"us{P}{q}",
                                    name=f"Usq{P}{q}") for q in range(4)]
                for q in range(4):
                    for cs in range(2):
                        dst = Usq[q][cs * 64:(cs + 1) * 64, :]
                        src = D2[cs * 64 + q * 16: cs * 64 + (q + 1) * 16,
                                 :].rearrange("j (k c) -> k j c", c=128)
                        c["io"].dma_start(
                            out=dst.rearrange("k (j c) -> k j c", c=128),
                            in_=src)
                st[P]["Usq"] = Usq

            def s7_alloc(P):
                c = cfg[P]
                st[P]["X01"] = c["s2"].tile([128, 4096], BF16, tag=f"s{P}2",
                                            name=f"X01_{P}")

            def s7_group(P, g):
                c = cfg[P]
                Usq = st[P]["Usq"][g // 2]
                X01 = st[P]["X01"]
                pt = ps.tile([128, 512], F32, tag="ps")
                for nn in range(8):
                    t = (8 * g + nn) % 16
                    nc.tensor.matmul(
                        pt[:, nn * 64:(nn + 1) * 64],
                        Usq[:, t * 128:(t + 1) * 128],
                        ics, start=True, stop=True,
                    )
                eng = nc.vector.tensor_copy if g % 4 == 0 else nc.scalar.copy
                eng(X01[:, g * 512:(g + 1) * 512], pt[:])

            def warm_pe(rhs, n, tag):
                # HAM bridge: the PE re-throttles to 1.2 GHz after ~3.4us of
                # low activity, and this kernel's matmul bursts sit just
                # under the busy-window between DMA waits, so the clock
                # stays cold (N=512 matmuls measure 427ns vs 215 warm).
                # Dummy matmuls chained by WAW on one PSUM tile keep the
                # activity monitor busy across the pivot gaps.  `rhs` ties
                # the group's start to the preceding stage's output.
                wt = ps.tile([128, 512], F32, tag="ps", name=f"warm{tag}")
                for _ in range(n):
                    nc.tensor.matmul(wt[:], khbd, rhs, start=True, stop=True)

            # ---- emission: pipe A leads, pipe B staggered ----
            s1_load(0)
            s1_load(1)
            nc.gpsimd.dma_start(out=weffA[:], in_=wa_d[:])
            nc.gpsimd.dma_start(out=weffB[:], in_=wb_d[:])
            s2_fh(0)
            p1_pivot(0)
            s2_fh(1)
            warm_pe(st[1]["T1p"][:, 0:512], 16, "a")
            s4_s5(0)
            p1_pivot(1)
            p2_pivot(0)
            s4_s5(1)
            s7_alloc(0)
            for g in range(8):
                s7_group(0, g)
            p2_pivot(1)
            warm_pe(st[1]["U2s"][:, 0:512], 12, "d")
            s7_alloc(1)
            X01A, X01B = st[0]["X01"], st[1]["X01"]

            # ---- S8 proj + LN, interleaved with s7(pipe B) per quarter ----
            Yq = [yr.tile([128, 2048], BF16, tag=f"yq{q}", name=f"Yq{q}")
                  for q in range(4)]

            for gg in range(4):
                s7_group(1, 2 * gg)
                s7_group(1, 2 * gg + 1)
                mvq = small.tile([128, 16], F32, tag=f"mv{gg}", name=f"mv{gg}")
                rstdq = small.tile([128, 8], F32, tag=f"rs{gg}", name=f"rs{gg}")
                nmrq = small.tile([128, 8], F32, tag=f"nm{gg}", name=f"nm{gg}")
                mvv = mvq[:].rearrange("p (t x) -> p t x", x=2)
                for tt in range(8):
                    t2 = gg * 8 + tt
                    pty = psy.tile([128, 256], F32, tag="psy", name=f"py{gg}{tt}")
                    nc.tensor.matmul(pty[:], X01A[:, t2 * 128:(t2 + 1) * 128],
                                     pjt[:, 0:256], start=True, stop=False)
                    nc.tensor.matmul(pty[:], X01B[:, t2 * 128:(t2 + 1) * 128],
                                     pjt[:, 256:512], start=False, stop=True)
                    stats = small.tile([128, 6], F32, tag="stats")
                    nc.vector.bn_stats(out=stats[:], in_=pty[:])
                    nc.vector.bn_aggr(out=mvq[:, tt * 2:(tt + 1) * 2],
                                      in_=stats[:])
                    nc.scalar.copy(Yq[gg][:, tt * 256:(tt + 1) * 256], pty[:])
                # std = sqrt(var + eps); rstd = 1/std; nmr = -mu*rstd
                nc.scalar.activation(out=rstdq[:], in_=mvv[:, :, 1],
                                     func=ACTF.Sqrt, bias=eps[:], scale=1.0)
                nc.vector.reciprocal(rstdq[:], rstdq[:])
                nc.vector.tensor_tensor(out=nmrq[:], in0=mvv[:, :, 0],
                                        in1=rstdq[:], op=ALU.mult)
                nc.vector.tensor_scalar_mul(nmrq[:], nmrq[:], -1.0)
                for tt in range(8):
                    ysl = slice(tt * 256, (tt + 1) * 256)
                    eng = nc.vector if tt == 7 else nc.gpsimd
                    eng.tensor_scalar(
                        out=Yq[gg][:, ysl], in0=Yq[gg][:, ysl],
                        scalar1=rstdq[:, tt: tt + 1],
                        scalar2=nmrq[:, tt: tt + 1],
                        op0=ALU.mult, op1=ALU.add,
                    )
                    if apply_gb:
                        nc.vector.tensor_mul(Yq[gg][:, ysl],
                                             Yq[gg][:, ysl], gt[:])
                        nc.gpsimd.tensor_add(Yq[gg][:, ysl],
                                             Yq[gg][:, ysl], bt[:])
                nc.sync.dma_start(out=y_d[:, gg * 2048:(gg + 1) * 2048],
                                  in_=Yq[gg][:])

    _split_multi_waits(nc)
    return nc


def _get_nc(apply_gb):
    key = bool(apply_gb)
    if key not in _NC_CACHE:
        _NC_CACHE[key] = _build_nc(key)
    return _NC_CACHE[key]


def _make_inputs(x, W_low, W_mid, W_high, proj_w, ln_g, ln_b):
    khbd, cs2, ICS = _host_matrices()

    W_eff = W_high[0].copy()
    W_eff[:32, :32] += W_mid[0]
    W_eff[:16, :16] += W_low[0]
    weffs = []
    for P in range(2):
        wr = W_eff[:, :, P * 128:(P + 1) * 128].transpose(1, 0, 2).reshape(64, 8192)
        weffs.append(np.ascontiguousarray(
            wr.reshape(64, 2, 4096).transpose(1, 0, 2).reshape(128, 4096)
            .astype(ml_dtypes.bfloat16)
        ))

    pjt = np.zeros((128, 512), ml_dtypes.bfloat16)
    pjt[:, :256] = proj_w.T[:128]
    pjt[:, 256:] = proj_w.T[128:]

    cst = np.concatenate(
        [np.asarray(khbd), np.asarray(cs2), np.asarray(ICS), pjt],
        axis=1).astype(ml_dtypes.bfloat16)

    gb = np.stack([ln_g, ln_b]).astype(np.float32)
    consts = {"cst": np.ascontiguousarray(cst),
              "weffa": weffs[0], "weffb": weffs[1], "gb": gb}

    in_maps = []
    for b in range(B):
        m = dict(consts)
        for P, name in ((0, "xra"), (1, "xrb")):
            xp = x[b].reshape(64, 64, 256)[:, :, P * 128:(P + 1) * 128]
            m[name] = np.ascontiguousarray(
                xp.reshape(64, 2, 32, 128).transpose(1, 0, 2, 3)
                .reshape(128, 4096).astype(ml_dtypes.bfloat16)
            )
        in_maps.append(m)
    return in_maps


def kernel(x, W_low, W_mid, W_high, proj_w, ln_g, ln_b):
    x = np.ascontiguousarray(np.asarray(x, dtype=np.float32))
    W_low = np.asarray(W_low, dtype=np.float32)
    W_mid = np.asarray(W_mid, dtype=np.float32)
    W_high = np.asarray(W_high, dtype=np.float32)
    proj_w = np.asarray(proj_w, dtype=np.float32)
    ln_g = np.asarray(ln_g, dtype=np.float32)
    ln_b = np.asarray(ln_b, dtype=np.float32)

    apply_gb = not (np.all(ln_g == 1.0) and np.all(ln_b == 0.0))
    in_maps = _make_inputs(x, W_low, W_mid, W_high, proj_w, ln_g, ln_b)
    nc = _get_nc(apply_gb)
    res = run_bass_kernel_spmd(nc, in_maps, core_ids=list(range(B)))

    out = np.empty((B, N, C), np.float32)
    for b in range(B):
        yc = np.asarray(res.results[b]["y"]).astype(np.float32)
        yc = yc.reshape(128, 32, 256).transpose(1, 0, 2).reshape(4096, 256)
        out[b] = yc.reshape(64, 64, 256).transpose(1, 0, 2).reshape(4096, 256)
    return out


# revision 47
# speedup vs baseline: 1.0733x; 1.0733x over previous
"""Trainium2 Bass kernel for nn_BandSplitDCTFilter.

Math: the reference's mirror-FFT DCT / band filter / inverse collapses to
    out_c = C1 (Z_c) C2^T - S1 (Z_c) S2^T,   Z_c = (A x_c A^T) .* W_eff_c
with A[k,j] = 2cos(pi k (2j+1)/128); C2/S2 carry the irfft half-spectrum
weights u_l and the 1/(4HW) scale; W_eff = pad(W_low)+pad(W_mid)+W_high
merges the three bands (they share the inverse basis under zero-padding).
Then y = x_out @ proj_w^T and LayerNorm.

Sharding: pure data-parallel, one sample per core (B=8 = 8 cores), small
weights replicated.

Final version (~107us HW vs the 121us two-pipe v1 and the 133us graded
baseline).  Changes over v1, driven by the measured DMA cost model
(every dma_start costs its ISSUING sequencer ~0.7us + ~1.7ns/descriptor
as a DIRECT2D instruction, plus ~30-44ns/descriptor on the 16 SDMA
queues; scattered 256B-run transfers are descriptor-bound):
  - pipe A keeps the sync queue; ALL of pipe B's DMAs move to the
    gpsimd queue so the scalar (ACT) sequencer never issues DMAs --
    in v1 pipe B's descriptor generation stole ~20us of ACT compute.
  - both pivots now store CONTIGUOUSLY to DRAM (cheap, few descriptors)
    and do the transpose on the gather-read side: p1 reloads per
    khh-half so s4 starts after half the gather, p2 reloads into four
    per-quarter Ustk tiles so s7/proj/LN pipeline against the gather.
    (SBUF->SBUF pivots would be ~2x cheaper still, but the walrus BIR
    verifier requires partition-crossing SBUF access patterns to be
    tile-anchored at offset 0, which the 32-row chunked sources here
    cannot satisfy -- hence DRAM-mediated exchanges.)
  - DMA instruction count ~60 -> ~36: packed consts (khbd|cs2|ics|pjt
    in one [128,832] tensor), one x load per pipe, merged pivot dumps.
  - LN smalls per group of 8; normalize runs 7/8 on gpsimd (tensor_
    scalar in SBUF) to unload the vector engine, which saturated the
    tail; X01 drains lean 3/4 onto scalar for the same reason.
  - PSUM split: 5-deep [128,512] ring for transform stages + 3-deep
    [128,256] ring for proj tiles.
"""

import os

os.environ.setdefault("JAX_PLATFORMS", "axon,cpu")

import numpy as np
import ml_dtypes

import bass_rust
import concourse.bass as bass
import concourse.mybir as mybir
from concourse.tile import TileContext, ScopedClock
from concourse.bass_utils import run_bass_kernel_spmd

# ---------------------------------------------------------------------------
# Workarounds: this container's walrus rejects >1 sync wait per instruction.
# ---------------------------------------------------------------------------

_wait_ctr = 0


def _split_multi_waits(nc, max_waits=1):
    global _wait_ctr
    for f in nc.m.functions:
        for bb in f.blocks:
            out = []
            dirty = False
            for ins in bb.instructions:
                si = ins.sync_info
                if si is not None and len(si.on_wait) > max_waits:
                    waits = list(si.on_wait)
                    for w in waits[:-max_waits]:
                        _wait_ctr += 1
                        nop = bass_rust.InstNoOp(name=f"I-waitsplit-{_wait_ctr}")
                        nop.engine = ins.engine
                        nop.sync_info = mybir.SyncInfo(on_wait=[w], on_update=[])
                        out.append(nop)
                    ins.sync_info = mybir.SyncInfo(
                        on_wait=waits[-max_waits:], on_update=list(si.on_update)
                    )
                    dirty = True
                out.append(ins)
            if dirty:
                bb.instructions = out


def _patched_drain_and_barrier(self, tick_clock, wait_clock):
    nc = self.nc
    probe = nc.sync.nop(nofuse=True)
    wait_clock.add_sem_waits(probe.ins, ScopedClock({None: tick_clock.global_clock}))
    si = probe.ins.sync_info
    waits = list(si.on_wait) if si is not None else []
    probe.ins.sync_info = mybir.SyncInfo(on_wait=waits[:1], on_update=[])
    name2sem = {s.name: s for s in self.sems.allocated().values()}
    for w in waits[1:]:
        nc.sync.nop(nofuse=True)._wait_ge(name2sem[w.ant_name], w.wait_value)
    nc.sync.drain()
    nc.all_engine_barrier()
    popped = nc._tile_sem_poison_stack.pop()
    assert popped is self._sem_poison
    nc.clear_and_free_semaphores(list(self.sems.allocated().values()))
    nc.all_engine_barrier()


TileContext._drain_and_barrier = _patched_drain_and_barrier

# ---------------------------------------------------------------------------

B, H, W, C = 8, 64, 64, 256
N = H * W
F32 = mybir.dt.float32
BF16 = mybir.dt.bfloat16
ALU = mybir.AluOpType
ACTF = mybir.ActivationFunctionType


def _host_matrices():
    k = np.arange(64)
    j = np.arange(64)
    ang = np.pi * k[:, None] * (2 * j[None, :] + 1) / 128.0
    A = 2.0 * np.cos(ang)
    u = np.where(k == 0, 1.0, 2.0)
    C1T = np.cos(ang)
    S1T = np.sin(ang)
    C2T = u[:, None] * np.cos(ang) / 16384.0
    S2T = u[:, None] * np.sin(ang) / 16384.0

    AT = A.T.astype(np.float32)                                   # [h, k]
    khbd = np.zeros((128, 128), np.float32)
    khbd[0:64, 0:64] = AT
    khbd[64:128, 64:128] = AT
    cs2_half = np.concatenate([C2T, S2T], axis=1)                 # [l, 128]
    cs2 = np.concatenate([cs2_half, cs2_half], axis=0)
    ICS = np.concatenate([C1T, -S1T], axis=0)
    return (khbd.astype(ml_dtypes.bfloat16),
            cs2.astype(ml_dtypes.bfloat16),
            np.ascontiguousarray(ICS.astype(ml_dtypes.bfloat16)))


_NC_CACHE = {}


def _build_nc(apply_gb):
    nc = bass.Bass(trn_type="TRN2")

    xa_d = nc.dram_tensor("xra", [128, 4096], BF16, kind="ExternalInput")
    xb_d = nc.dram_tensor("xrb", [128, 4096], BF16, kind="ExternalInput")
    cst_d = nc.dram_tensor("cst", [128, 832], BF16, kind="ExternalInput")
    wa_d = nc.dram_tensor("weffa", [128, 4096], BF16, kind="ExternalInput")
    wb_d = nc.dram_tensor("weffb", [128, 4096], BF16, kind="ExternalInput")
    gb_d = nc.dram_tensor("gb", [2, 256], F32, kind="ExternalInput")
    y_d = nc.dram_tensor("y", [128, 8192], BF16, kind="ExternalOutput")

    with TileContext(nc) as tc:
        with (
            tc.tile_pool(name="consts", bufs=1) as consts,
            tc.tile_pool(name="wfA", bufs=1) as wfA,
            tc.tile_pool(name="wfB", bufs=1) as wfB,
            tc.tile_pool(name="sA1", bufs=1) as sA1,
            tc.tile_pool(name="sA2", bufs=1) as sA2,
            tc.tile_pool(name="sA3", bufs=1) as sA3,
            tc.tile_pool(name="sB1", bufs=1) as sB1,
            tc.tile_pool(name="sB2", bufs=1) as sB2,
            tc.tile_pool(name="sB3", bufs=1) as sB3,
            tc.tile_pool(name="zA", bufs=1) as zA,
            tc.tile_pool(name="zB", bufs=1) as zB,
            tc.tile_pool(name="yr", bufs=1) as yr,
            tc.tile_pool(name="dramp", bufs=1, space="DRAM") as dramp,
            tc.tile_pool(name="ps", bufs=5, space="PSUM") as ps,
            tc.tile_pool(name="psy", bufs=3, space="PSUM") as psy,
            tc.tile_pool(name="small", bufs=16) as small,
        ):
            # ---- constants (one packed dma on gpsimd) ----
            cst = consts.tile([128, 832], BF16, tag="cst")
            nc.gpsimd.dma_start(out=cst[:], in_=cst_d[:])
            khbd = cst[:, 0:128]
            cs2 = cst[:, 128:256]
            ics = cst[:, 256:320]
            pjt = cst[:, 320:832]
            eps = consts.tile([128, 1], F32, tag="eps")
            nc.vector.memset(eps[:], 1e-5)
            i256 = consts.tile([128, 1], F32, tag="i256")
            nc.vector.memset(i256[:], 1.0 / 256.0)
            weffA = wfA.tile([128, 4096], BF16, tag="wfA")
            weffB = wfB.tile([128, 4096], BF16, tag="wfB")
            # NOTE: weff dmas are issued AFTER the x loads (below) -- the
            # gpsimd ring is FIFO, and 2MB of weff ahead of pipe B's x load
            # delayed s2(B) by ~5us (weff isn't needed until s4).
            if apply_gb:
                gt = consts.tile([128, 256], F32, tag="gt")
                bt = consts.tile([128, 256], F32, tag="bt")
                gb_ap = gb_d.ap()
                g_b = bass.AP(tensor=gb_ap.tensor, offset=0, ap=[[0, 128], [1, 256]])
                b_b = bass.AP(tensor=gb_ap.tensor, offset=256, ap=[[0, 128], [1, 256]])
                nc.gpsimd.dma_start(out=gt[:], in_=g_b)
                nc.gpsimd.dma_start(out=bt[:], in_=b_b)

            cfg = {
                0: dict(x_d=xa_d, io=nc.sync, s1=sA1, s2=sA2, s3=sA3, zp=zA),
                1: dict(x_d=xb_d, io=nc.gpsimd, s1=sB1, s2=sB2, s3=sB3, zp=zB),
            }
            st = {0: {}, 1: {}}

            def s1_load(P):
                c = cfg[P]
                X = c["s1"].tile([128, 4096], BF16, tag=f"s{P}1")
                c["io"].dma_start(out=X[:], in_=c["x_d"][:])
                st[P]["X"] = X

            def s2_fh(P):
                # T1[(wh,k),(w32,c)] = blockdiag(A^T)^T @ X  (K=128 full)
                c = cfg[P]
                X = st[P]["X"]
                T1p = c["s2"].tile([128, 4096], BF16, tag=f"s{P}2")
                for j in range(8):
                    sl = slice(j * 512, (j + 1) * 512)
                    pt = ps.tile([128, 512], F32, tag="ps")
                    nc.tensor.matmul(pt[:], khbd, X[:, sl],
                                     start=True, stop=True)
                    eng = nc.vector.tensor_copy if j % 2 == 0 else nc.scalar.copy
                    eng(T1p[:, sl], pt[:])
                st[P]["T1p"] = T1p

            def p1_pivot(P):
                # Contiguous dump of T1p rows; the kh<->w exchange happens on
                # the READ side so s4 can start per khh-half.
                c = cfg[P]
                T1p = st[P]["T1p"]
                D1 = dramp.tile([128, 4096], BF16, tag=f"d1{P}", name=f"D1_{P}")
                c["io"].dma_start(out=D1[:], in_=T1p[:])
                T2h = [c["s3"].tile([128, 2048], BF16, tag=f"t2{P}{h}",
                                    name=f"T2h{P}{h}") for h in range(2)]
                for h in range(2):
                    for ks in range(2):
                        for w1 in range(2):
                            dst = T2h[h][ks * 64 + w1 * 32:
                                         ks * 64 + w1 * 32 + 32, :]
                            r0 = w1 * 64 + ks * 32 + h * 16
                            src = D1[r0: r0 + 16, :].rearrange(
                                "k (w c) -> w k c", c=128)
                            # pipe A: halve the serial descriptor-gen on the
                            # sync sequencer (ACT's is idle in this window)
                            eng = (nc.scalar if (P == 0 and ks == 1)
                                   else c["io"])
                            eng.dma_start(
                                out=dst.rearrange("w (k c) -> w k c", c=128),
                                in_=src)
                st[P]["T2h"] = T2h

            def s4_s5(P):
                c = cfg[P]
                T2h = st[P]["T2h"]
                weff = weffA if P == 0 else weffB
                Zp = c["zp"].tile([128, 4096], BF16, tag=f"z{P}")
                for j in range(8):
                    sl = slice(j * 512, (j + 1) * 512)
                    pt = ps.tile([128, 512], F32, tag="ps")
                    nc.tensor.matmul(pt[:], khbd,
                                     T2h[j // 4][:, (j % 4) * 512:
                                                 (j % 4 + 1) * 512],
                                     start=True, stop=True)
                    nc.vector.tensor_mul(Zp[:, sl], pt[:], weff[:, sl])
                U2s = c["s3"].tile([128, 8192], BF16, tag=f"s{P}3")
                for j in range(16):
                    off = 64 * (j // 8)
                    sl = slice((j % 8) * 512, (j % 8 + 1) * 512)
                    pt = ps.tile([128, 512], F32, tag="ps")
                    nc.tensor.matmul(pt[:], cs2[off:off + 64, :],
                                     Zp[off:off + 64, sl], start=True, stop=True)
                    dsl = slice(j * 512, (j + 1) * 512)
                    eng = nc.vector.tensor_copy if j % 2 == 0 else nc.scalar.copy
                    eng(U2s[:, dsl], pt[:])
                st[P]["U2s"] = U2s

            def p2_pivot(P):
                # Contiguous store of U2s; the j<->kh exchange happens on the
                # READ side (gather loads) so it pipelines with s7/proj.
                c = cfg[P]
                U2s = st[P]["U2s"]
                D2 = dramp.tile([128, 8192], BF16, tag=f"d2{P}", name=f"D2_{P}")
                c["io"].dma_start(out=D2[:], in_=U2s[:])
                Usq = [c["s1"].tile([128, 2048], BF16, tag=f"us{P}{q}",
                                    name=f"Usq{P}{q}") for q in range(4)]
                for q in range(4):
                    for cs in range(2):
                        dst = Usq[q][cs * 64:(cs + 1) * 64, :]
                        src = D2[cs * 64 + q * 16: cs * 64 + (q + 1) * 16,
                                 :].rearrange("j (k c) -> k j c", c=128)
                        c["io"].dma_start(
                            out=dst.rearrange("k (j c) -> k j c", c=128),
                            in_=src)
                st[P]["Usq"] = Usq

            def s7_alloc(P):
                c = cfg[P]
                st[P]["X01"] = c["s2"].tile([128, 4096], BF16, tag=f"s{P}2",
                                            name=f"X01_{P}")

            def s7_group(P, g):
                c = cfg[P]
                Usq = st[P]["Usq"][g // 2]
                X01 = st[P]["X01"]
                pt = ps.tile([128, 512], F32, tag="ps")
                for nn in range(8):
                    t = (8 * g + nn) % 16
                    nc.tensor.matmul(
                        pt[:, nn * 64:(nn + 1) * 64],
                        Usq[:, t * 128:(t + 1) * 128],
                        ics, start=True, stop=True,
                    )
                eng = nc.vector.tensor_copy if g % 4 == 0 else nc.scalar.copy
                eng(X01[:, g * 512:(g + 1) * 512], pt[:])

            def warm_pe(rhs, n, tag):
                # HAM bridge: the PE re-throttles to 1.2 GHz after ~3.4us of
                # low activity, and this kernel's matmul bursts sit just
                # under the busy-window between DMA waits, so the clock
                # stays cold (N=512 matmuls measure 427ns vs 215 warm).
                # Dummy matmuls chained by WAW on one PSUM tile keep the
                # activity monitor busy across the pivot gaps.  `rhs` ties
                # the group's start to the preceding stage's output.
                wt = ps.tile([128, 512], F32, tag="ps", name=f"warm{tag}")
                for _ in range(n):
                    nc.tensor.matmul(wt[:], khbd, rhs, start=True, stop=True)

            # ---- emission: pipe A leads, pipe B staggered ----
            s1_load(0)
            s1_load(1)
            nc.gpsimd.dma_start(out=weffA[:], in_=wa_d[:])
            nc.gpsimd.dma_start(out=weffB[:], in_=wb_d[:])
            s2_fh(0)
            p1_pivot(0)
            s2_fh(1)
            warm_pe(st[1]["T1p"][:, 0:512], 8, "a")
            s4_s5(0)
            p1_pivot(1)
            p2_pivot(0)
            s4_s5(1)
            s7_alloc(0)
            for g in range(8):
                s7_group(0, g)
            p2_pivot(1)
            s7_alloc(1)
            X01A, X01B = st[0]["X01"], st[1]["X01"]

            # ---- S8 proj + LN, interleaved with s7(pipe B) per quarter ----
            Yq = [yr.tile([128, 2048], BF16, tag=f"yq{q}", name=f"Yq{q}")
                  for q in range(4)]

            for gg in range(4):
                s7_group(1, 2 * gg)
                s7_group(1, 2 * gg + 1)
                mvq = small.tile([128, 16], F32, tag=f"mv{gg}", name=f"mv{gg}")
                rstdq = small.tile([128, 8], F32, tag=f"rs{gg}", name=f"rs{gg}")
                nmrq = small.tile([128, 8], F32, tag=f"nm{gg}", name=f"nm{gg}")
                mvv = mvq[:].rearrange("p (t x) -> p t x", x=2)
                for tt in range(8):
                    t2 = gg * 8 + tt
                    pty = psy.tile([128, 256], F32, tag="psy", name=f"py{gg}{tt}")
                    nc.tensor.matmul(pty[:], X01A[:, t2 * 128:(t2 + 1) * 128],
                                     pjt[:, 0:256], start=True, stop=False)
                    nc.tensor.matmul(pty[:], X01B[:, t2 * 128:(t2 + 1) * 128],
                                     pjt[:, 256:512], start=False, stop=True)
                    stats = small.tile([128, 6], F32, tag="stats")
                    nc.vector.bn_stats(out=stats[:], in_=pty[:])
                    nc.vector.bn_aggr(out=mvq[:, tt * 2:(tt + 1) * 2],
                                      in_=stats[:])
                    nc.scalar.copy(Yq[gg][:, tt * 256:(tt + 1) * 256], pty[:])
                # std = sqrt(var + eps); rstd = 1/std; nmr = -mu*rstd
                nc.scalar.activation(out=rstdq[:], in_=mvv[:, :, 1],
                                     func=ACTF.Sqrt, bias=eps[:], scale=1.0)
                nc.vector.reciprocal(rstdq[:], rstdq[:])
                nc.vector.tensor_tensor(out=nmrq[:], in0=mvv[:, :, 0],
                                        in1=rstdq[:], op=ALU.mult)
                nc.vector.tensor_scalar_mul(nmrq[:], nmrq[:], -1.0)
                for tt in range(8):
                    ysl = slice(tt * 256, (tt + 1) * 256)
                    eng = nc.vector if tt == 7 else nc.gpsimd
                    eng.tensor_scalar(
                        out=Yq[gg][:, ysl], in0=Yq[gg][:, ysl],
                        scalar1=rstdq[:, tt: tt + 1],
                        scalar2=nmrq[:, tt: tt + 1],
                        op0=ALU.mult, op1=ALU.add,
                    )
                    if apply_gb:
                        nc.vector.tensor_mul(Yq[gg][:, ysl],
                                             Yq[gg][:, ysl], gt[:])
                        nc.gpsimd.tensor_add(Yq[gg][:, ysl],
                                             Yq[gg][:, ysl], bt[:])
                nc.sync.dma_start(out=y_d[:, gg * 2048:(gg + 1) * 2048],
                                  in_=Yq[gg][:])

    _split_multi_waits(nc)
    return nc


def _get_nc(apply_gb):
    key = bool(apply_gb)
    if key not in _NC_CACHE:
        _NC_CACHE[key] = _build_nc(key)
    return _NC_CACHE[key]


def _make_inputs(x, W_low, W_mid, W_high, proj_w, ln_g, ln_b):
    khbd, cs2, ICS = _host_matrices()

    W_eff = W_high[0].copy()
    W_eff[:32, :32] += W_mid[0]
    W_eff[:16, :16] += W_low[0]
    weffs = []
    for P in range(2):
        wr = W_eff[:, :, P * 128:(P + 1) * 128].transpose(1, 0, 2).reshape(64, 8192)
        weffs.append(np.ascontiguousarray(
            wr.reshape(64, 2, 4096).transpose(1, 0, 2).reshape(128, 4096)
            .astype(ml_dtypes.bfloat16)
        ))

    pjt = np.zeros((128, 512), ml_dtypes.bfloat16)
    pjt[:, :256] = proj_w.T[:128]
    pjt[:, 256:] = proj_w.T[128:]

    cst = np.concatenate(
        [np.asarray(khbd), np.asarray(cs2), np.asarray(ICS), pjt],
        axis=1).astype(ml_dtypes.bfloat16)

    gb = np.stack([ln_g, ln_b]).astype(np.float32)
    consts = {"cst": np.ascontiguousarray(cst),
              "weffa": weffs[0], "weffb": weffs[1], "gb": gb}

    in_maps = []
    for b in range(B):
        m = dict(consts)
        for P, name in ((0, "xra"), (1, "xrb")):
            xp = x[b].reshape(64, 64, 256)[:, :, P * 128:(P + 1) * 128]
            m[name] = np.ascontiguousarray(
                xp.reshape(64, 2, 32, 128).transpose(1, 0, 2, 3)
                .reshape(128, 4096).astype(ml_dtypes.bfloat16)
            )
        in_maps.append(m)
    return in_maps


def kernel(x, W_low, W_mid, W_high, proj_w, ln_g, ln_b):
    x = np.ascontiguousarray(np.asarray(x, dtype=np.float32))
    W_low = np.asarray(W_low, dtype=np.float32)
    W_mid = np.asarray(W_mid, dtype=np.float32)
    W_high = np.asarray(W_high, dtype=np.float32)
    proj_w = np.asarray(proj_w, dtype=np.float32)
    ln_g = np.asarray(ln_g, dtype=np.float32)
    ln_b = np.asarray(ln_b, dtype=np.float32)

    apply_gb = not (np.all(ln_g == 1.0) and np.all(ln_b == 0.0))
    in_maps = _make_inputs(x, W_low, W_mid, W_high, proj_w, ln_g, ln_b)
    nc = _get_nc(apply_gb)
    res = run_bass_kernel_spmd(nc, in_maps, core_ids=list(range(B)))

    out = np.empty((B, N, C), np.float32)
    for b in range(B):
        yc = np.asarray(res.results[b]["y"]).astype(np.float32)
        yc = yc.reshape(128, 32, 256).transpose(1, 0, 2).reshape(4096, 256)
        out[b] = yc.reshape(64, 64, 256).transpose(1, 0, 2).reshape(4096, 256)
    return out
